# revision 99
# baseline (speedup 1.0000x reference)
"""Trainium2 Bass kernel for CaiT talking-heads attention.

B=8 batch, N=1024 tokens, DIM=512, 8 heads x 64. Data-parallel: one batch
element per NeuronCore (8 cores).

Per-core algorithm:
  x^T via PE transpose (is_transpose mode, bf16)
  Q^T = w_q^T x^T, K^T = w_k^T x^T (feature-major), V = x w_v (token-major)
  for g in heads:                       # mixed-pre head index
    Qs_g = Q^T scaled rows by mix_pre[h(c),g]/8   (folds mix_pre + scale)
    S'^T_g = K^T.T-contracted vs Qs_g   # [j, i] tiles, K=512 contraction
    P_g = exp(S'^T_g)                   # softmax w/o max-sub (|S'| ~ < 6)
    V'_g = V * mix_post[g, head(col)]   (folds mix_post)
    out += (P_g @ V'_g) / rowsum(P_g)   # rowsum via ones-matmul piggyback
  y = out @ w_out + b_out  (out PE-transposed so it feeds lhsT directly)

Dtypes: x/w_q/w_k/w_v/w_out stream in as bf16; both big matmul groups run
fp8e4m3 DoubleRow (2 k-tiles per instruction at 0.5 cyc/row = 4x fp32r):
scores use a 3-term compensated form (Qs8.Kh + Qs8.Kl + Qsl.Kh, where Kh/Kl
is an exact fp8 hi/lo pair of K/2 and Qsl the fp8 residual of Qs) so fp8
costs ~no accuracy there; PV uses exp->fp8 P directly against an exact fp8
hi/lo pair of V/2 (the mix_post column scale is pulled out of the matmul and
applied to the psum result, keeping V head-independent; the ones=0.5
normalizer absorbs the V/2 scale). OUT accumulates f32r; y streams out bf16.
Measured end-to-end rel err 1.58e-2 vs the f32 reference (gate 2e-2).

Scheduling: weight DMAs ride the Act DGE queue in need-order while x
streams in block chunks on the SP/Pool queues; x^T transposes start as each
chunk lands and interleave into the Q-projection chains; Qs scaling runs on
the Act engine, V' scaling on the Pool engine; the V projection slots between
the first head's two score blocks; OUT transposes and the output projection
interleave into the last head's PV chains; mpo replicates on-chip via
partition_broadcast; b_out is added on the host.
"""

import numpy as np
import ml_dtypes

import concourse.bacc as bacc
import concourse.mybir as mybir
from concourse.bass_utils import run_bass_kernel_spmd
from concourse.masks import make_identity
from concourse.tile import TileContext

P = 128
N = 1024
DIM = 512
H = 8
DH = 64
F32 = mybir.dt.float32
F32R = mybir.dt.float32r
BF16 = mybir.dt.bfloat16
F8 = mybir.dt.float8e4

IB = N // P    # 8 token blocks
CC = DIM // P  # 4 feature chunks
NCORES = 8


def build_bass():
    nc = bacc.Bacc("TRN2")

    # x arrives host-shuffled to [P, IB, DIM] (partition-major) so paired
    # token blocks stream in single DMAs with matching AP iteration order
    x_d = nc.dram_tensor("x", [P, IB, DIM], BF16, kind="ExternalInput")
    # wq/wk host-interleaved row-wise: halves the DMA count on Act
    wqk_d = nc.dram_tensor("wqk", [DIM, 2, DIM], BF16, kind="ExternalInput")
    wv_d = nc.dram_tensor("wv", [DIM, DIM], BF16, kind="ExternalInput")
    wout_d = nc.dram_tensor("wout", [DIM, DIM], BF16, kind="ExternalInput")
    # mp[p, cc*8+g] = mix_pre[(cc*128+p)//64, g] / 8
    mp_d = nc.dram_tensor("mp", [P, CC * H], F32, kind="ExternalInput")
    # mpo_s[0, h*512 + g*64+d] = mix_post[h, g]; replicated on-chip
    mpo_d = nc.dram_tensor("mpo", [1, H * DIM], F32R, kind="ExternalInput")
    # b_out is added on the host: y here is OUT @ w_out only
    y_d = nc.dram_tensor("y", [N, DIM], BF16, kind="ExternalOutput")

    with TileContext(nc) as tc:
        with (
            tc.tile_pool(name="persist", bufs=1) as pp,
            tc.tile_pool(name="ph01", bufs=1) as p01,
            tc.tile_pool(name="ph2", bufs=2) as p2,
            tc.tile_pool(name="ph34", bufs=1) as p34,
            tc.tile_pool(name="ps2", bufs=5, space="PSUM") as psp,
            tc.tile_pool(name="psr", bufs=2, space="PSUM") as psr,
        ):
            QT = pp.tile([P, CC, N], F32R)   # QT[p,cc,i] = q[i, cc*128+p]
            # K streams into an exact fp8 hi/lo pair (at 1/2 scale; Qs
            # carries the matching x2) so the score matmuls can run
            # fp8 DoubleRow at 2x the fp32r rate with only Qs quant error
            KTh = pp.tile([P, CC, N], F8)
            KTl = pp.tile([P, CC, N], F8)
            # V also streams into an exact fp8 hi/lo pair (1/2 scale); the
            # per-head mix_post column scale is applied to the PV psum
            # result instead, so V stays head-independent
            Vh = pp.tile([P, IB, DIM], F8)   # V[p,jb,gd] = v[jb*128+p, gd]
            Vl = pp.tile([P, IB, DIM], F8)
            OUT = pp.tile([P, IB, DIM], F32R)
            mp = pp.tile([P, CC * H], F32)
            mpo_s = pp.tile([1, H * DIM], F32R)
            mpo = pp.tile([P, H, DIM], F32R)
            wout = pp.tile([P, CC, DIM], BF16)

            # ---- DMA issue. All transfers share one serial HBM pipe, so
            # issue order ~ need order.  x streams in chunks alternating
            # SP/Pool queues (chunk DMAs overlap their 900ns completion
            # semaphores); wq/wk/mp ride the Act queue, which stays short so
            # Act's SEQ frees up early for the Qs scaling. ----
            xsb = []
            chunks = [(0, 1, nc.sync), (1, 3, nc.gpsimd), (3, 5, nc.sync),
                      (5, 7, nc.gpsimd), (7, 8, nc.sync)]
            for lo, hi, eng in chunks:
                t = p01.tile([P, hi - lo, DIM], BF16, tag=f"xsb{lo}")
                eng.dma_start(t[:], x_d[:, lo:hi, :])
                for b in range(lo, hi):
                    xsb.append(t[:, b - lo, :])
            wqk = p01.tile([P, CC, 2, DIM], BF16)
            wq = wqk[:, :, 0, :]
            wk = wqk[:, :, 1, :]
            wv = p01.tile([P, CC, DIM], BF16)
            for c in range(CC):
                nc.scalar.dma_start(
                    wqk[:, c, :, :], wqk_d[c * P:(c + 1) * P, :, :])
            nc.scalar.dma_start(mp[:], mp_d[:])
            for c in range(CC):
                nc.sync.dma_start(wv[:, c, :], wv_d[c * P:(c + 1) * P, :])
            nc.sync.dma_start(mpo_s[:], mpo_d[:])
            for c in range(CC):
                nc.scalar.dma_start(wout[:, c, :], wout_d[c * P:(c + 1) * P, :])
            # replicate the tiny broadcast operand on the idle Pool engine
            nc.gpsimd.partition_broadcast(mpo[:], mpo_s[:])

            ident0 = pp.tile([P, P], F32)
            make_identity(nc, ident0)
            identr = pp.tile([P, P], F32R)
            nc.vector.tensor_copy(identr[:], ident0[:])
            ident = identr[:]
            identb = pp.tile([P, P], BF16)
            nc.vector.tensor_copy(identb[:], ident0[:])
            ones0 = pp.tile([P, 2, 8], F8)
            nc.vector.memset(ones0, 0.5)
            ones = ones0[:]


            xT = p01.tile([P, CC, N], BF16)  # xT[p,fc,i] = x[i, fc*128+p]

            def transpose_block(b):
                # 4 transposes into one bf16 psum tile (disjoint regions),
                # drained by a single wide copy
                pt = psp.tile([P, CC, P], BF16, tag="pst", bufs=1)
                for fc in range(CC):
                    nc.tensor.matmul(
                        pt[:, fc, :], xsb[b][:, fc * P:(fc + 1) * P], identb,
                        is_transpose=True,
                        start=(fc == 0), stop=(fc == CC - 1),
                        skip_group_check=True,
                    )
                nc.vector.tensor_copy(xT[:, :, b * P:(b + 1) * P], pt[:])

            def proj_T(dst, w, ih, interleave={}):
                isl = slice(ih * 512, (ih + 1) * 512)
                for cc in range(CC):
                    pq = psp.tile([P, DIM], F32, tag="ps")
                    for fc in range(CC):
                        nc.tensor.matmul(
                            pq, w[:, fc, cc * P:(cc + 1) * P], xT[:, fc, isl],
                            start=(fc == 0), stop=(fc == CC - 1),
                        )
                    nc.vector.tensor_copy(dst[:, cc, isl], pq)
                    for tb in interleave.get(cc, ()):
                        transpose_block(tb)

            def proj_K(ih):
                isl = slice(ih * 512, (ih + 1) * 512)
                for cc in range(CC):
                    pq = psp.tile([P, DIM], F32, tag="ps")
                    for fc in range(CC):
                        nc.tensor.matmul(
                            pq, wk[:, fc, cc * P:(cc + 1) * P], xT[:, fc, isl],
                            start=(fc == 0), stop=(fc == CC - 1),
                        )
                    # Kh is a pure scaled copy -> Act engine (idle in this
                    # window); DVE keeps only the Kl residual
                    nc.scalar.mul(KTh[:, cc, isl], pq, 0.5)
                    nc.vector.scalar_tensor_tensor(
                        out=KTl[:, cc, isl], in0=pq, scalar=0.5, in1=KTh[:, cc, isl],
                        op0=mybir.AluOpType.mult,
                        op1=mybir.AluOpType.subtract,
                    )

            for b in (0, 1, 2, 3):
                transpose_block(b)
            proj_T(QT, wq, 0, interleave={0: (6, 7), 1: (4, 5)})
            proj_T(QT, wq, 1)
            # head 0's Qs/Qsl emit before the K work so the DVE residual
            # is long done when the first score chains need it
            qs0 = p2.tile([P, CC, N], F8, tag="qs", bufs=3)
            for cc in range(CC):
                nc.scalar.mul(
                    qs0[:, cc, :], QT[:, cc, :], mp[:, cc * H:cc * H + 1])
            proj_K(0)
            # qsl0 lands between the two K-split batches on DVE, matching
            # the order the first head's score chains consume them
            qsl0 = p2.tile([P, CC, N], F8, tag="qsl", bufs=3)
            for cc in range(CC):
                nc.vector.scalar_tensor_tensor(
                    out=qsl0[:, cc, :], in0=QT[:, cc, :],
                    scalar=mp[:, cc * H:cc * H + 1], in1=qs0[:, cc, :],
                    op0=mybir.AluOpType.mult,
                    op1=mybir.AluOpType.subtract,
                )
            proj_K(1)

            def emit_v():
                for jb in range(IB):
                    pv = psp.tile([P, DIM], F32, tag="ps")
                    for fc in range(CC):
                        nc.tensor.matmul(
                            pv, xT[:, fc, jb * P:(jb + 1) * P], wv[:, fc, :],
                            start=(fc == 0), stop=(fc == CC - 1),
                        )
                    nc.scalar.mul(Vh[:, jb, :], pv, 0.5)
                    nc.vector.scalar_tensor_tensor(
                        out=Vl[:, jb, :], in0=pv, scalar=0.5,
                        in1=Vh[:, jb, :],
                        op0=mybir.AluOpType.mult,
                        op1=mybir.AluOpType.subtract,
                    )

            OT = p34.tile([P, CC, N], BF16)

            def out_transpose(bs):
                # OUT[:, b, :] -> OT[:, gc, b-block] once head g=7 done.
                # All four transposes land in one psum bank (disjoint column
                # ranges, accumulate-into-zeroed), drained by a single copy.
                for b in bs:
                    pt = psp.tile([P, CC, P], F32, tag="pst", bufs=1)
                    ptr = pt.bitcast(F32R)
                    for gc in range(CC):
                        nc.tensor.matmul(
                            ptr[:, gc, :],
                            OUT[:, b, gc * P:(gc + 1) * P], ident,
                            is_transpose=True,
                            start=(gc == 0), stop=(gc == CC - 1),
                            skip_group_check=True,
                        )
                    dst = OT[:, :, b * P:(b + 1) * P]
                    if b % 2 == 0:
                        nc.vector.tensor_copy(dst, ptr[:])
                    else:
                        nc.scalar.copy(dst, ptr[:])

            def emit_proj(bs, py34):
                for b in bs:
                    py = psp.tile([P, DIM], F32, tag="ps")
                    for gc in range(CC):
                        nc.tensor.matmul(
                            py, OT[:, gc, b * P:(b + 1) * P], wout[:, gc, :],
                            start=(gc == 0), stop=(gc == CC - 1),
                        )
                    ysb = py34.tile([P, DIM], BF16, tag="y")
                    # bias is added on the host; alternate copy engines +
                    # DGE queues so the last blocks drain in parallel
                    if b % 2 == 0:
                        nc.vector.tensor_copy(ysb[:], py)
                        nc.sync.dma_start(y_d[b * P:(b + 1) * P, :], ysb)
                    else:
                        nc.scalar.copy(ysb[:], py)
                        nc.scalar.dma_start(y_d[b * P:(b + 1) * P, :], ysb)

            # ---- per mixed-head scores+softmax+PV ----
            with tc.tile_pool(name="y34", bufs=4) as py34:
                for h in range(H):
                    # Qs on the Act engine (fp8 hi) with a DVE residual (lo):
                    # 3-term compensated scores have ~no fp8 error
                    if h == 0:
                        Qs, Qsl = qs0, qsl0
                    else:
                        Qs = p2.tile([P, CC, N], F8, tag="qs", bufs=3)
                        for cc in range(CC):
                            nc.scalar.mul(
                                Qs[:, cc, :], QT[:, cc, :],
                                mp[:, cc * H + h:cc * H + h + 1],
                            )
                        Qsl = p2.tile([P, CC, N], F8, tag="qsl", bufs=3)
                        for cc in range(CC):
                            nc.vector.scalar_tensor_tensor(
                                out=Qsl[:, cc, :], in0=QT[:, cc, :],
                                scalar=mp[:, cc * H + h:cc * H + h + 1],
                                in1=Qs[:, cc, :],
                                op0=mybir.AluOpType.mult,
                                op1=mybir.AluOpType.subtract,
                            )
                    PTs = []
                    for ih in range(2):
                        isl = slice(ih * 512, (ih + 1) * 512)
                        PT = p2.tile([P, IB, 512], F8, tag="pt", bufs=4)
                        PTs.append(PT)
                        # software-pipelined: part1 uses only Qs8 (Act),
                        # part2 (the Qsl residual terms, DVE) trails by a
                        # few chains so the PE never waits on the residual
                        pss = {}

                        def sc_part1(jb):
                            ps = psp.tile([P, DIM], F32, tag="ps")
                            pss[jb] = ps
                            steps = [(KTh, 0), (KTh, 1), (KTl, 0), (KTl, 1)]
                            for k, (KX, c2) in enumerate(steps):
                                nc.tensor.matmul(
                                    ps,
                                    KX[:, 2 * c2:2 * c2 + 2,
                                       jb * P:(jb + 1) * P],
                                    Qs[:, 2 * c2:2 * c2 + 2, isl],
                                    start=(k == 0), stop=False,
                                    perf_mode=mybir.MatmulPerfMode.DoubleRow,
                                )

                        def sc_part2(jb):
                            ps = pss.pop(jb)
                            for c2 in range(2):
                                nc.tensor.matmul(
                                    ps,
                                    KTh[:, 2 * c2:2 * c2 + 2,
                                        jb * P:(jb + 1) * P],
                                    Qsl[:, 2 * c2:2 * c2 + 2, isl],
                                    start=False, stop=(c2 == 1),
                                    perf_mode=mybir.MatmulPerfMode.DoubleRow,
                                )
                            nc.scalar.activation(
                                PT[:, jb, :], ps,
                                mybir.ActivationFunctionType.Exp,
                            )

                        depth = 2 if ih == 0 else 2
                        for jb in range(IB):
                            sc_part1(jb)
                            if jb >= depth:
                                sc_part2(jb - depth)
                        for jb in range(IB - depth, IB):
                            sc_part2(jb)
                        if h == 0 and ih == 0:
                            # V projection slots in here, hidden under the
                            # first score block
                            emit_v()
                    def pv_rowsum(ibs):
                        PT = PTs[ibs // 4]
                        il = ibs % 4
                        pr = psr.tile([P, 8], F32, tag="pr")
                        # rowsum chain first: its reciprocal clears the DVE
                        # queue while the PV chain still runs on the PE
                        for jp in range(4):
                            nc.tensor.matmul(
                                pr,
                                PT[:, 2 * jp:2 * jp + 2,
                                   il * P:(il + 1) * P],
                                ones,
                                start=(jp == 0), stop=(jp == 3),
                                perf_mode=mybir.MatmulPerfMode.DoubleRow,
                            )
                        rr = p2.tile([P, 1], F32, tag="rr", bufs=8)
                        nc.vector.reciprocal(rr, pr[:, 0:1])
                        return rr

                    def pv_po(ibs, rr, csl=slice(0, DIM)):
                        PT = PTs[ibs // 4]
                        il = ibs % 4
                        ncols = csl.stop - csl.start
                        po = psp.tile([P, ncols], F32, tag="ps")
                        for k, VX in enumerate((Vh, Vl)):
                            for jp in range(4):
                                nc.tensor.matmul(
                                    po,
                                    PT[:, 2 * jp:2 * jp + 2,
                                       il * P:(il + 1) * P],
                                    VX[:, 2 * jp:2 * jp + 2, csl],
                                    start=(k == 0 and jp == 0),
                                    stop=(k == 1 and jp == 3),
                                    perf_mode=mybir.MatmulPerfMode.DoubleRow,
                                )
                        # one DVE op applies both the softmax normalizer
                        # (scalar slot) and the mix_post column scale (tensor
                        # slot) to the psum; Pool then does a plain SBUF add
                        if h == 0:
                            nc.vector.scalar_tensor_tensor(
                                out=OUT[:, ibs, csl], in0=po, scalar=rr,
                                in1=mpo[:, 0, csl],
                                op0=mybir.AluOpType.mult,
                                op1=mybir.AluOpType.mult,
                            )
                        else:
                            tmp = p2.tile([P, ncols], F32, tag="tmp", bufs=8)
                            nc.vector.scalar_tensor_tensor(
                                out=tmp[:], in0=po, scalar=rr,
                                in1=mpo[:, h, csl],
                                op0=mybir.AluOpType.mult,
                                op1=mybir.AluOpType.mult,
                            )
                            nc.gpsimd.tensor_add(
                                out=OUT[:, ibs, csl], in0=tmp,
                                in1=OUT[:, ibs, csl],
                            )

                    def pv_chain(ibs):
                        rr = pv_rowsum(ibs)
                        pv_po(ibs, rr)

                    if h < H - 1:
                        for ibs in range(IB):
                            pv_chain(ibs)
                    else:
                        # last head: thread OUT transposes (T) and output
                        # projections (P) between the PV chains (C) so only
                        # the last block's T/P trails the final chain
                        for step in ("C0 C1 C2 T0 C3 T1 P0 C4 T2 P1 C5 T3 "
                                     "P2 C6 T4 P3 T5 P4 P5 T6").split():
                            b = int(step[1])
                            if step[0] == "C":
                                pv_chain(b)
                            elif step[0] == "T":
                                out_transpose([b])
                            else:
                                emit_proj([b], py34)
                        # block 7 runs in column halves so its transposes,
                        # OT copies, projection, and DMA pipeline tightly
                        rr7 = pv_rowsum(7)
                        pv_po(7, rr7, slice(0, 256))
                        pv_po(7, rr7, slice(256, DIM))
                        pt7 = psp.tile([P, CC, P], F32, tag="pst", bufs=1)
                        pt7r = pt7.bitcast(F32R)

                        def t7(gcs, last):
                            for gc in gcs:
                                nc.tensor.matmul(
                                    pt7r[:, gc, :],
                                    OUT[:, 7, gc * P:(gc + 1) * P], ident,
                                    is_transpose=True,
                                    start=(gc == 0),
                                    stop=(last and gc == gcs[-1]),
                                    skip_group_check=True,
                                )

                        t7([0, 1], False)
                        nc.scalar.copy(
                            OT[:, 0:2, 7 * P:8 * P], pt7r[:, 0:2, :])
                        emit_proj([6], py34)
                        t7([2, 3], True)
                        nc.vector.tensor_copy(
                            OT[:, 2:4, 7 * P:8 * P], pt7r[:, 2:4, :])
                        # final projection: half-width ysb copies and DMAs
                        # drain on both engines/queues in parallel
                        py = psp.tile([P, DIM], F32, tag="ps")
                        for gc in range(CC):
                            nc.tensor.matmul(
                                py, OT[:, gc, 7 * P:8 * P], wout[:, gc, :],
                                start=(gc == 0), stop=(gc == CC - 1),
                            )
                        ysb = py34.tile([P, DIM], BF16, tag="y")
                        nc.vector.tensor_copy(ysb[:, 0:256], py[:, 0:256])
                        nc.vector.tensor_copy(ysb[:, 256:DIM], py[:, 256:DIM])
                        nc.sync.dma_start(y_d[7 * P:N, 0:256], ysb[:, 0:256])
                        nc.sync.dma_start(
                            y_d[7 * P:N, 256:DIM], ysb[:, 256:DIM])

    nc.finalize()
    return nc


_NC_CACHE = None
TRACE = False
LAST_RESULT = None


def kernel(x, w_q, w_kv, mix_pre, mix_post, w_out, b_out):
    global _NC_CACHE
    x = np.asarray(x, np.float32)
    w_q = np.asarray(w_q, np.float32)
    w_kv = np.asarray(w_kv, np.float32)
    mix_pre = np.asarray(mix_pre, np.float32)
    mix_post = np.asarray(mix_post, np.float32)
    w_out = np.asarray(w_out, np.float32)
    b_out = np.asarray(b_out, np.float32)

    bf = ml_dtypes.bfloat16
    wqk = np.ascontiguousarray(np.stack(
        [w_q.astype(bf), w_kv[:, :DIM].astype(bf)], axis=1))
    w_v = np.ascontiguousarray(w_kv[:, DIM:].astype(bf))

    w_o8 = np.ascontiguousarray(w_out.astype(bf))

    # mp[p, cc*8+g] = mix_pre[head of channel cc*128+p, g] * (1/sqrt(64))
    ch = (np.arange(DIM) // DH)  # head of channel
    mp = np.zeros((P, CC * H), np.float32)
    for cc in range(CC):
        for g in range(H):
            mp[:, cc * H + g] = mix_pre[ch[cc * P:(cc + 1) * P], g] * 0.25
    # mpo_s[0, h*512+col] = mix_post[h, col//64]
    mpo_s = np.ascontiguousarray(
        np.repeat(mix_post, DH, axis=1).reshape(1, H * DIM).astype(np.float32)
    )

    if _NC_CACHE is None:
        _NC_CACHE = build_bass()
    nc = _NC_CACHE

    base = {
        "wqk": wqk, "wv": w_v, "wout": w_o8,
        "mp": mp, "mpo": mpo_s,
    }
    in_maps = [
        dict(base, x=np.ascontiguousarray(
            x[b].astype(bf).reshape(IB, P, DIM).transpose(1, 0, 2)))
        for b in range(NCORES)
    ]
    global LAST_RESULT
    res = run_bass_kernel_spmd(
        nc, in_maps, core_ids=list(range(NCORES)), trace=TRACE,
        trace_cores=list(range(NCORES)) if TRACE else None,
    )
    LAST_RESULT = res
    out = np.stack(
        [np.asarray(res.results[b]["y"], dtype=np.float32)
         for b in range(NCORES)], axis=0)
    return out + b_out[None, None, :]


# revision 100
# speedup vs baseline: 1.0057x; 1.0057x over previous
"""Trainium2 Bass kernel for CaiT talking-heads attention.

B=8 batch, N=1024 tokens, DIM=512, 8 heads x 64. Data-parallel: one batch
element per NeuronCore (8 cores).

Per-core algorithm:
  x^T via PE transpose (is_transpose mode, bf16)
  Q^T = w_q^T x^T, K^T = w_k^T x^T (feature-major), V = x w_v (token-major)
  for g in heads:                       # mixed-pre head index
    Qs_g = Q^T scaled rows by mix_pre[h(c),g]/8   (folds mix_pre + scale)
    S'^T_g = K^T.T-contracted vs Qs_g   # [j, i] tiles, K=512 contraction
    P_g = exp(S'^T_g)                   # softmax w/o max-sub (|S'| ~ < 6)
    V'_g = V * mix_post[g, head(col)]   (folds mix_post)
    out += (P_g @ V'_g) / rowsum(P_g)   # rowsum via ones-matmul piggyback
  y = out @ w_out + b_out  (out PE-transposed so it feeds lhsT directly)

Dtypes: x/w_q/w_k/w_v/w_out stream in as bf16; both big matmul groups run
fp8e4m3 DoubleRow (2 k-tiles per instruction at 0.5 cyc/row = 4x fp32r):
scores use a 3-term compensated form (Qs8.Kh + Qs8.Kl + Qsl.Kh, where Kh/Kl
is an exact fp8 hi/lo pair of K/2 and Qsl the fp8 residual of Qs) so fp8
costs ~no accuracy there; PV uses exp->fp8 P directly against an exact fp8
hi/lo pair of V/2 (the mix_post column scale is pulled out of the matmul and
applied to the psum result, keeping V head-independent; the ones=0.5
normalizer absorbs the V/2 scale). OUT accumulates f32r; y streams out bf16.
Measured end-to-end rel err 1.58e-2 vs the f32 reference (gate 2e-2).

Scheduling: weight DMAs ride the Act DGE queue in need-order while x
streams in block chunks on the SP/Pool queues; x^T transposes start as each
chunk lands and interleave into the Q-projection chains; Qs scaling runs on
the Act engine, V' scaling on the Pool engine; the V projection slots between
the first head's two score blocks; OUT transposes and the output projection
interleave into the last head's PV chains; mpo replicates on-chip via
partition_broadcast; b_out is added on the host.
"""

import numpy as np
import ml_dtypes

import concourse.bacc as bacc
import concourse.mybir as mybir
from concourse.bass_utils import run_bass_kernel_spmd
from concourse.masks import make_identity
from concourse.tile import TileContext

P = 128
N = 1024
DIM = 512
H = 8
DH = 64
F32 = mybir.dt.float32
F32R = mybir.dt.float32r
BF16 = mybir.dt.bfloat16
F8 = mybir.dt.float8e4

IB = N // P    # 8 token blocks
CC = DIM // P  # 4 feature chunks
NCORES = 8


def build_bass():
    nc = bacc.Bacc("TRN2")

    # x arrives host-shuffled to [P, IB, DIM] (partition-major) so paired
    # token blocks stream in single DMAs with matching AP iteration order
    x_d = nc.dram_tensor("x", [P, IB, DIM], BF16, kind="ExternalInput")
    wq_d = nc.dram_tensor("wq", [DIM, DIM], BF16, kind="ExternalInput")
    wk_d = nc.dram_tensor("wk", [DIM, DIM], BF16, kind="ExternalInput")
    wv_d = nc.dram_tensor("wv", [DIM, DIM], BF16, kind="ExternalInput")
    wout_d = nc.dram_tensor("wout", [DIM, DIM], BF16, kind="ExternalInput")
    # mp[p, cc*8+g] = mix_pre[(cc*128+p)//64, g] / 8
    mp_d = nc.dram_tensor("mp", [P, CC * H], F32, kind="ExternalInput")
    # mpo_s[0, h*512 + g*64+d] = mix_post[h, g]; replicated on-chip
    mpo_d = nc.dram_tensor("mpo", [1, H * DIM], F32R, kind="ExternalInput")
    # b_out is added on the host: y here is OUT @ w_out only
    y_d = nc.dram_tensor("y", [N, DIM], BF16, kind="ExternalOutput")

    with TileContext(nc) as tc:
        with (
            tc.tile_pool(name="persist", bufs=1) as pp,
            tc.tile_pool(name="ph01", bufs=1) as p01,
            tc.tile_pool(name="ph2", bufs=2) as p2,
            tc.tile_pool(name="ph34", bufs=1) as p34,
            tc.tile_pool(name="ps2", bufs=5, space="PSUM") as psp,
            tc.tile_pool(name="psr", bufs=2, space="PSUM") as psr,
        ):
            QT = pp.tile([P, CC, N], F32R)   # QT[p,cc,i] = q[i, cc*128+p]
            # K streams into an exact fp8 hi/lo pair (at 1/2 scale; Qs
            # carries the matching x2) so the score matmuls can run
            # fp8 DoubleRow at 2x the fp32r rate with only Qs quant error
            KTh = pp.tile([P, CC, N], F8)
            KTl = pp.tile([P, CC, N], F8)
            # V also streams into an exact fp8 hi/lo pair (1/2 scale); the
            # per-head mix_post column scale is applied to the PV psum
            # result instead, so V stays head-independent
            Vh = pp.tile([P, IB, DIM], F8)   # V[p,jb,gd] = v[jb*128+p, gd]
            Vl = pp.tile([P, IB, DIM], F8)
            OUT = pp.tile([P, IB, DIM], F32R)
            mp = pp.tile([P, CC * H], F32)
            mpo_s = pp.tile([1, H * DIM], F32R)
            mpo = pp.tile([P, H, DIM], F32R)
            wout = pp.tile([P, CC, DIM], BF16)

            # ---- DMA issue. All transfers share one serial HBM pipe, so
            # issue order ~ need order.  x streams in chunks alternating
            # SP/Pool queues (chunk DMAs overlap their 900ns completion
            # semaphores); wq/wk/mp ride the Act queue, which stays short so
            # Act's SEQ frees up early for the Qs scaling. ----
            xsb = []
            chunks = [(0, 1, nc.sync), (1, 3, nc.gpsimd), (3, 5, nc.sync),
                      (5, 7, nc.gpsimd), (7, 8, nc.sync)]
            for lo, hi, eng in chunks:
                t = p01.tile([P, hi - lo, DIM], BF16, tag=f"xsb{lo}")
                eng.dma_start(t[:], x_d[:, lo:hi, :])
                for b in range(lo, hi):
                    xsb.append(t[:, b - lo, :])
            wq = p01.tile([P, CC, DIM], BF16)
            wk = p01.tile([P, CC, DIM], BF16)
            wv = p01.tile([P, CC, DIM], BF16)
            for c in range(CC):
                nc.scalar.dma_start(wq[:, c, :], wq_d[c * P:(c + 1) * P, :])
            for c in range(CC):
                nc.scalar.dma_start(wk[:, c, :], wk_d[c * P:(c + 1) * P, :])
            nc.scalar.dma_start(mp[:], mp_d[:])
            for c in range(CC):
                nc.sync.dma_start(wv[:, c, :], wv_d[c * P:(c + 1) * P, :])
            nc.sync.dma_start(mpo_s[:], mpo_d[:])
            for c in range(CC):
                nc.scalar.dma_start(wout[:, c, :], wout_d[c * P:(c + 1) * P, :])
            # replicate the tiny broadcast operand on the idle Pool engine
            nc.gpsimd.partition_broadcast(mpo[:], mpo_s[:])

            ident0 = pp.tile([P, P], F32)
            make_identity(nc, ident0)
            identr = pp.tile([P, P], F32R)
            nc.vector.tensor_copy(identr[:], ident0[:])
            ident = identr[:]
            identb = pp.tile([P, P], BF16)
            nc.vector.tensor_copy(identb[:], ident0[:])
            ones0 = pp.tile([P, 2, 8], F8)
            nc.vector.memset(ones0, 0.5)
            ones = ones0[:]


            xT = p01.tile([P, CC, N], BF16)  # xT[p,fc,i] = x[i, fc*128+p]

            def transpose_block(b):
                # 4 transposes into one bf16 psum tile (disjoint regions),
                # drained by a single wide copy
                pt = psp.tile([P, CC, P], BF16, tag="pst", bufs=1)
                for fc in range(CC):
                    nc.tensor.matmul(
                        pt[:, fc, :], xsb[b][:, fc * P:(fc + 1) * P], identb,
                        is_transpose=True,
                        start=(fc == 0), stop=(fc == CC - 1),
                        skip_group_check=True,
                    )
                nc.vector.tensor_copy(xT[:, :, b * P:(b + 1) * P], pt[:])

            def proj_T(dst, w, ih, interleave={}):
                isl = slice(ih * 512, (ih + 1) * 512)
                for cc in range(CC):
                    pq = psp.tile([P, DIM], F32, tag="ps")
                    for fc in range(CC):
                        nc.tensor.matmul(
                            pq, w[:, fc, cc * P:(cc + 1) * P], xT[:, fc, isl],
                            start=(fc == 0), stop=(fc == CC - 1),
                        )
                    nc.vector.tensor_copy(dst[:, cc, isl], pq)
                    for tb in interleave.get(cc, ()):
                        transpose_block(tb)

            def proj_K(ih):
                isl = slice(ih * 512, (ih + 1) * 512)
                for cc in range(CC):
                    pq = psp.tile([P, DIM], F32, tag="ps")
                    for fc in range(CC):
                        nc.tensor.matmul(
                            pq, wk[:, fc, cc * P:(cc + 1) * P], xT[:, fc, isl],
                            start=(fc == 0), stop=(fc == CC - 1),
                        )
                    # Kh is a pure scaled copy -> Act engine (idle in this
                    # window); DVE keeps only the Kl residual
                    nc.scalar.mul(KTh[:, cc, isl], pq, 0.5)
                    nc.vector.scalar_tensor_tensor(
                        out=KTl[:, cc, isl], in0=pq, scalar=0.5, in1=KTh[:, cc, isl],
                        op0=mybir.AluOpType.mult,
                        op1=mybir.AluOpType.subtract,
                    )

            for b in (0, 1, 2, 3):
                transpose_block(b)
            proj_T(QT, wq, 0, interleave={0: (6, 7), 1: (4, 5)})
            proj_T(QT, wq, 1)
            # head 0's Qs/Qsl emit before the K work so the DVE residual
            # is long done when the first score chains need it
            qs0 = p2.tile([P, CC, N], F8, tag="qs", bufs=3)
            for cc in range(CC):
                nc.scalar.mul(
                    qs0[:, cc, :], QT[:, cc, :], mp[:, cc * H:cc * H + 1])
            proj_K(0)
            # qsl0 lands between the two K-split batches on DVE, matching
            # the order the first head's score chains consume them
            qsl0 = p2.tile([P, CC, N], F8, tag="qsl", bufs=3)
            for cc in range(CC):
                nc.vector.scalar_tensor_tensor(
                    out=qsl0[:, cc, :], in0=QT[:, cc, :],
                    scalar=mp[:, cc * H:cc * H + 1], in1=qs0[:, cc, :],
                    op0=mybir.AluOpType.mult,
                    op1=mybir.AluOpType.subtract,
                )
            proj_K(1)

            def emit_v():
                for jb in range(IB):
                    pv = psp.tile([P, DIM], F32, tag="ps")
                    for fc in range(CC):
                        nc.tensor.matmul(
                            pv, xT[:, fc, jb * P:(jb + 1) * P], wv[:, fc, :],
                            start=(fc == 0), stop=(fc == CC - 1),
                        )
                    nc.scalar.mul(Vh[:, jb, :], pv, 0.5)
                    nc.vector.scalar_tensor_tensor(
                        out=Vl[:, jb, :], in0=pv, scalar=0.5,
                        in1=Vh[:, jb, :],
                        op0=mybir.AluOpType.mult,
                        op1=mybir.AluOpType.subtract,
                    )

            OT = p34.tile([P, CC, N], BF16)

            def out_transpose(bs):
                # OUT[:, b, :] -> OT[:, gc, b-block] once head g=7 done.
                # All four transposes land in one psum bank (disjoint column
                # ranges, accumulate-into-zeroed), drained by a single copy.
                for b in bs:
                    pt = psp.tile([P, CC, P], F32, tag="pst", bufs=1)
                    ptr = pt.bitcast(F32R)
                    for gc in range(CC):
                        nc.tensor.matmul(
                            ptr[:, gc, :],
                            OUT[:, b, gc * P:(gc + 1) * P], ident,
                            is_transpose=True,
                            start=(gc == 0), stop=(gc == CC - 1),
                            skip_group_check=True,
                        )
                    dst = OT[:, :, b * P:(b + 1) * P]
                    if b % 2 == 0:
                        nc.vector.tensor_copy(dst, ptr[:])
                    else:
                        nc.scalar.copy(dst, ptr[:])

            def emit_proj(bs, py34):
                for b in bs:
                    py = psp.tile([P, DIM], F32, tag="ps")
                    for gc in range(CC):
                        nc.tensor.matmul(
                            py, OT[:, gc, b * P:(b + 1) * P], wout[:, gc, :],
                            start=(gc == 0), stop=(gc == CC - 1),
                        )
                    ysb = py34.tile([P, DIM], BF16, tag="y")
                    # bias is added on the host; alternate copy engines +
                    # DGE queues so the last blocks drain in parallel
                    if b % 2 == 0:
                        nc.vector.tensor_copy(ysb[:], py)
                        nc.sync.dma_start(y_d[b * P:(b + 1) * P, :], ysb)
                    else:
                        nc.scalar.copy(ysb[:], py)
                        nc.scalar.dma_start(y_d[b * P:(b + 1) * P, :], ysb)

            # ---- per mixed-head scores+softmax+PV ----
            with tc.tile_pool(name="y34", bufs=4) as py34:
                for h in range(H):
                    # Qs on the Act engine (fp8 hi) with a DVE residual (lo):
                    # 3-term compensated scores have ~no fp8 error
                    if h == 0:
                        Qs, Qsl = qs0, qsl0
                    else:
                        Qs = p2.tile([P, CC, N], F8, tag="qs", bufs=3)
                        for cc in range(CC):
                            nc.scalar.mul(
                                Qs[:, cc, :], QT[:, cc, :],
                                mp[:, cc * H + h:cc * H + h + 1],
                            )
                        Qsl = p2.tile([P, CC, N], F8, tag="qsl", bufs=3)
                        for cc in range(CC):
                            nc.vector.scalar_tensor_tensor(
                                out=Qsl[:, cc, :], in0=QT[:, cc, :],
                                scalar=mp[:, cc * H + h:cc * H + h + 1],
                                in1=Qs[:, cc, :],
                                op0=mybir.AluOpType.mult,
                                op1=mybir.AluOpType.subtract,
                            )
                    PTs = []
                    for ih in range(2):
                        isl = slice(ih * 512, (ih + 1) * 512)
                        PT = p2.tile([P, IB, 512], F8, tag="pt", bufs=4)
                        PTs.append(PT)
                        # software-pipelined: part1 uses only Qs8 (Act),
                        # part2 (the Qsl residual terms, DVE) trails by a
                        # few chains so the PE never waits on the residual
                        pss = {}

                        def sc_part1(jb):
                            ps = psp.tile([P, DIM], F32, tag="ps")
                            pss[jb] = ps
                            steps = [(KTh, 0), (KTh, 1), (KTl, 0), (KTl, 1)]
                            for k, (KX, c2) in enumerate(steps):
                                nc.tensor.matmul(
                                    ps,
                                    KX[:, 2 * c2:2 * c2 + 2,
                                       jb * P:(jb + 1) * P],
                                    Qs[:, 2 * c2:2 * c2 + 2, isl],
                                    start=(k == 0), stop=False,
                                    perf_mode=mybir.MatmulPerfMode.DoubleRow,
                                )

                        def sc_part2(jb):
                            ps = pss.pop(jb)
                            for c2 in range(2):
                                nc.tensor.matmul(
                                    ps,
                                    KTh[:, 2 * c2:2 * c2 + 2,
                                        jb * P:(jb + 1) * P],
                                    Qsl[:, 2 * c2:2 * c2 + 2, isl],
                                    start=False, stop=(c2 == 1),
                                    perf_mode=mybir.MatmulPerfMode.DoubleRow,
                                )
                            nc.scalar.activation(
                                PT[:, jb, :], ps,
                                mybir.ActivationFunctionType.Exp,
                            )

                        depth = 2 if ih == 0 else 2
                        for jb in range(IB):
                            sc_part1(jb)
                            if jb >= depth:
                                sc_part2(jb - depth)
                        for jb in range(IB - depth, IB):
                            sc_part2(jb)
                        if h == 0 and ih == 0:
                            # V projection slots in here, hidden under the
                            # first score block
                            emit_v()
                    def pv_rowsum(ibs):
                        PT = PTs[ibs // 4]
                        il = ibs % 4
                        pr = psr.tile([P, 8], F32, tag="pr")
                        # rowsum chain first: its reciprocal clears the DVE
                        # queue while the PV chain still runs on the PE
                        for jp in range(4):
                            nc.tensor.matmul(
                                pr,
                                PT[:, 2 * jp:2 * jp + 2,
                                   il * P:(il + 1) * P],
                                ones,
                                start=(jp == 0), stop=(jp == 3),
                                perf_mode=mybir.MatmulPerfMode.DoubleRow,
                            )
                        rr = p2.tile([P, 1], F32, tag="rr", bufs=8)
                        nc.vector.reciprocal(rr, pr[:, 0:1])
                        return rr

                    def pv_po(ibs, rr, csl=slice(0, DIM)):
                        PT = PTs[ibs // 4]
                        il = ibs % 4
                        ncols = csl.stop - csl.start
                        po = psp.tile([P, ncols], F32, tag="ps")
                        for k, VX in enumerate((Vh, Vl)):
                            for jp in range(4):
                                nc.tensor.matmul(
                                    po,
                                    PT[:, 2 * jp:2 * jp + 2,
                                       il * P:(il + 1) * P],
                                    VX[:, 2 * jp:2 * jp + 2, csl],
                                    start=(k == 0 and jp == 0),
                                    stop=(k == 1 and jp == 3),
                                    perf_mode=mybir.MatmulPerfMode.DoubleRow,
                                )
                        # one DVE op applies both the softmax normalizer
                        # (scalar slot) and the mix_post column scale (tensor
                        # slot) to the psum; Pool then does a plain SBUF add
                        if h == 0:
                            nc.vector.scalar_tensor_tensor(
                                out=OUT[:, ibs, csl], in0=po, scalar=rr,
                                in1=mpo[:, 0, csl],
                                op0=mybir.AluOpType.mult,
                                op1=mybir.AluOpType.mult,
                            )
                        else:
                            tmp = p2.tile([P, ncols], F32, tag="tmp", bufs=8)
                            nc.vector.scalar_tensor_tensor(
                                out=tmp[:], in0=po, scalar=rr,
                                in1=mpo[:, h, csl],
                                op0=mybir.AluOpType.mult,
                                op1=mybir.AluOpType.mult,
                            )
                            nc.gpsimd.tensor_add(
                                out=OUT[:, ibs, csl], in0=tmp,
                                in1=OUT[:, ibs, csl],
                            )

                    def pv_chain(ibs):
                        rr = pv_rowsum(ibs)
                        pv_po(ibs, rr)

                    if h < H - 1:
                        for ibs in range(IB):
                            pv_chain(ibs)
                    else:
                        # last head: thread OUT transposes (T) and output
                        # projections (P) between the PV chains (C) so only
                        # the last block's T/P trails the final chain
                        for step in ("C0 C1 C2 T0 C3 T1 P0 C4 T2 P1 C5 T3 "
                                     "P2 C6 T4 P3 T5 P4 P5 T6").split():
                            b = int(step[1])
                            if step[0] == "C":
                                pv_chain(b)
                            elif step[0] == "T":
                                out_transpose([b])
                            else:
                                emit_proj([b], py34)
                        # block 7 runs in column halves so its transposes,
                        # OT copies, projection, and DMA pipeline tightly
                        rr7 = pv_rowsum(7)
                        pv_po(7, rr7, slice(0, 256))
                        pv_po(7, rr7, slice(256, DIM))
                        pt7 = psp.tile([P, CC, P], F32, tag="pst", bufs=1)
                        pt7r = pt7.bitcast(F32R)

                        def t7(gcs, last):
                            for gc in gcs:
                                nc.tensor.matmul(
                                    pt7r[:, gc, :],
                                    OUT[:, 7, gc * P:(gc + 1) * P], ident,
                                    is_transpose=True,
                                    start=(gc == 0),
                                    stop=(last and gc == gcs[-1]),
                                    skip_group_check=True,
                                )

                        t7([0, 1], False)
                        nc.scalar.copy(
                            OT[:, 0:2, 7 * P:8 * P], pt7r[:, 0:2, :])
                        emit_proj([6], py34)
                        t7([2, 3], True)
                        nc.vector.tensor_copy(
                            OT[:, 2:4, 7 * P:8 * P], pt7r[:, 2:4, :])
                        # final projection: half-width ysb copies and DMAs
                        # drain on both engines/queues in parallel
                        py = psp.tile([P, DIM], F32, tag="ps")
                        for gc in range(CC):
                            nc.tensor.matmul(
                                py, OT[:, gc, 7 * P:8 * P], wout[:, gc, :],
                                start=(gc == 0), stop=(gc == CC - 1),
                            )
                        ysb = py34.tile([P, DIM], BF16, tag="y")
                        nc.vector.tensor_copy(ysb[:, 0:256], py[:, 0:256])
                        nc.vector.tensor_copy(ysb[:, 256:DIM], py[:, 256:DIM])
                        nc.sync.dma_start(y_d[7 * P:N, 0:256], ysb[:, 0:256])
                        nc.sync.dma_start(
                            y_d[7 * P:N, 256:DIM], ysb[:, 256:DIM])

    nc.finalize()
    return nc


_NC_CACHE = None
TRACE = False
LAST_RESULT = None


def kernel(x, w_q, w_kv, mix_pre, mix_post, w_out, b_out):
    global _NC_CACHE
    x = np.asarray(x, np.float32)
    w_q = np.asarray(w_q, np.float32)
    w_kv = np.asarray(w_kv, np.float32)
    mix_pre = np.asarray(mix_pre, np.float32)
    mix_post = np.asarray(mix_post, np.float32)
    w_out = np.asarray(w_out, np.float32)
    b_out = np.asarray(b_out, np.float32)

    bf = ml_dtypes.bfloat16
    w_k = np.ascontiguousarray(w_kv[:, :DIM].astype(bf))
    w_v = np.ascontiguousarray(w_kv[:, DIM:].astype(bf))
    w_q8 = np.ascontiguousarray(w_q.astype(bf))
    w_o8 = np.ascontiguousarray(w_out.astype(bf))

    # mp[p, cc*8+g] = mix_pre[head of channel cc*128+p, g] * (1/sqrt(64))
    ch = (np.arange(DIM) // DH)  # head of channel
    mp = np.zeros((P, CC * H), np.float32)
    for cc in range(CC):
        for g in range(H):
            mp[:, cc * H + g] = mix_pre[ch[cc * P:(cc + 1) * P], g] * 0.25
    # mpo_s[0, h*512+col] = mix_post[h, col//64]
    mpo_s = np.ascontiguousarray(
        np.repeat(mix_post, DH, axis=1).reshape(1, H * DIM).astype(np.float32)
    )

    if _NC_CACHE is None:
        _NC_CACHE = build_bass()
    nc = _NC_CACHE

    base = {
        "wq": w_q8, "wk": w_k, "wv": w_v, "wout": w_o8,
        "mp": mp, "mpo": mpo_s,
    }
    in_maps = [
        dict(base, x=np.ascontiguousarray(
            x[b].astype(bf).reshape(IB, P, DIM).transpose(1, 0, 2)))
        for b in range(NCORES)
    ]
    global LAST_RESULT
    res = run_bass_kernel_spmd(
        nc, in_maps, core_ids=list(range(NCORES)), trace=TRACE,
        trace_cores=list(range(NCORES)) if TRACE else None,
    )
    LAST_RESULT = res
    out = np.stack(
        [np.asarray(res.results[b]["y"], dtype=np.float32)
         for b in range(NCORES)], axis=0)
    return out + b_out[None, None, :]
